# revision 13
# baseline (speedup 1.0000x reference)
"""AdaptiveCenterLoss on 8 TRN2 NeuronCores.

loss = mean_i ||features[i] - centers[labels[i]]||^2
     with B=131072, D=256, C=1000.

Strategy (data-parallel, memory-bound):
  - shard the batch 8 ways (16384 rows/core); replicate the centers table
  - per core: partition p owns rows [p*128, (p+1)*128) of its shard
  - indirect-DMA gather of centers rows: one [128,1]-index call per slot
    (the HW DGE consumes exactly one index per partition per call),
    writing into column slices of a [128, G*256] group tile
  - per group of G slots: DVE subtract, ACT square + row-sum accumulate
  - each core outputs [128, NT] partial sums; host sums and divides by B
"""

import numpy as np

import concourse.bacc as bacc
import concourse.bass as bass
import concourse.mybir as mybir
import concourse.tile as tile
from concourse.bass_utils import run_bass_kernel_spmd

B, D, C = 131072, 256, 1000
N_CORES = 8
B_CORE = B // N_CORES  # 16384 rows per core
P = 128                # SBUF partitions
SLOTS = B_CORE // P    # 128 rows per partition
G = 16                 # slots per compute group
NT = SLOTS // G        # 8 groups

USE_ACT_SQUARE = True

_nc_cache = None


def _build():
    global _nc_cache
    if _nc_cache is not None:
        return _nc_cache
    nc = bacc.Bacc()
    feats = nc.declare_dram_parameter(
        "features", [B_CORE, D], mybir.dt.float32, isOutput=False
    )
    labels = nc.declare_dram_parameter(
        "labels", [P, SLOTS], mybir.dt.int32, isOutput=False
    )
    centers = nc.declare_dram_parameter(
        "centers", [C, D], mybir.dt.float32, isOutput=False
    )
    out = nc.declare_dram_parameter("out", [P, NT], mybir.dt.float32, isOutput=True)

    # partition p, slot j <- feature row p*SLOTS + j
    fview = feats[:].rearrange("(p j) d -> p j d", p=P)

    with tile.TileContext(nc) as tc:
        with (
            tc.tile_pool(name="lab", bufs=1) as lab_pool,
            tc.tile_pool(name="f", bufs=3) as f_pool,
            tc.tile_pool(name="c", bufs=3) as c_pool,
            tc.tile_pool(name="sq", bufs=2) as sq_pool,
            tc.tile_pool(name="acc", bufs=1) as acc_pool,
        ):
            lab = lab_pool.tile([P, SLOTS], mybir.dt.int32)
            nc.sync.dma_start(out=lab[:], in_=labels[:])
            acc = acc_pool.tile([P, NT], mybir.dt.float32)
            for t in range(NT):
                f_t = f_pool.tile([P, G * D], mybir.dt.float32)
                nc.sync.dma_start(
                    out=f_t[:].rearrange("p (g d) -> p g d", g=G),
                    in_=fview[:, t * G : (t + 1) * G, :],
                )
                c_t = c_pool.tile([P, G * D], mybir.dt.float32)
                for g in range(G):
                    j = t * G + g
                    nc.gpsimd.indirect_dma_start(
                        out=c_t[:, g * D : (g + 1) * D],
                        out_offset=None,
                        in_=centers[:],
                        in_offset=bass.IndirectOffsetOnAxis(
                            ap=lab[:, j : j + 1], axis=0
                        ),
                    )
                nc.vector.tensor_tensor(
                    out=c_t[:], in0=f_t[:], in1=c_t[:], op=mybir.AluOpType.subtract
                )
                if USE_ACT_SQUARE:
                    nc.scalar.activation(
                        out=c_t[:],
                        in_=c_t[:],
                        func=mybir.ActivationFunctionType.Square,
                        accum_out=acc[:, t : t + 1],
                    )
                else:
                    sq_t = sq_pool.tile([P, G * D], mybir.dt.float32)
                    nc.vector.tensor_tensor(
                        out=sq_t[:], in0=c_t[:], in1=c_t[:], op=mybir.AluOpType.mult
                    )
                    nc.scalar.activation(
                        out=sq_t[:],
                        in_=sq_t[:],
                        func=mybir.ActivationFunctionType.Copy,
                        accum_out=acc[:, t : t + 1],
                    )
            nc.sync.dma_start(out=out[:], in_=acc[:])
    nc.finalize()
    _nc_cache = nc
    return nc


def _in_maps(features, centers, labels):
    features = np.ascontiguousarray(np.asarray(features), dtype=np.float32)
    centers = np.ascontiguousarray(np.asarray(centers), dtype=np.float32)
    labels = np.asarray(labels).astype(np.int32)
    maps = []
    for c in range(N_CORES):
        fs = np.ascontiguousarray(features[c * B_CORE : (c + 1) * B_CORE])
        ls = np.ascontiguousarray(
            labels[c * B_CORE : (c + 1) * B_CORE].reshape(P, SLOTS)
        )
        maps.append({"features": fs, "labels": ls, "centers": centers})
    return maps


def run(features, centers, labels, trace=False):
    """Run on 8 cores; returns (loss_scalar, BassKernelResults)."""
    nc = _build()
    res = run_bass_kernel_spmd(
        nc, _in_maps(features, centers, labels), core_ids=list(range(N_CORES)),
        trace=trace,
    )
    total = 0.0
    for r in res.results:
        total += float(np.asarray(r["out"]).astype(np.float64).sum())
    return np.float32(total / B), res


def kernel(features, centers, labels):
    loss, _ = run(features, centers, labels)
    return loss


# revision 14
# speedup vs baseline: 3.0928x; 3.0928x over previous
"""AdaptiveCenterLoss on 8 TRN2 NeuronCores.

loss = mean_i ||features[i] - centers[labels[i]]||^2
     with B=131072, D=256, C=1000.

Strategy (data-parallel, memory-bound):
  - host-side, sort rows by label and pack them into 16-row blocks, each
    block sharing one label; partial blocks are padded with rows equal to
    that class's center (contributing exactly 0 to the sum)
  - shard the padded blocks across 8 cores x 128 partitions; per tile
    (one block per partition), ONE [128,1]-index indirect DMA gathers the
    128 needed center rows (the per-descriptor DGE cost is the bottleneck
    of any per-row gather on this HW, so one gather per 16-row block
    instead of per row is ~16x cheaper)
  - per tile: DVE subtract (center broadcast across the 16 slots via a
    stride-0 AP), ACT square + row-sum accumulate
  - each core outputs [128, NT] partial sums; host sums and divides by B
"""

import numpy as np

import concourse.bacc as bacc
import concourse.bass as bass
import concourse.mybir as mybir
import concourse.tile as tile
from concourse.bass_utils import run_bass_kernel_spmd

B, D, C = 131072, 256, 1000
N_CORES = 8
P = 128   # SBUF partitions
S = 16    # rows (slots) per block

_nc_cache = {}


def _build(nt):
    """Build the per-core graph for `nt` tiles (nt blocks per partition)."""
    if nt in _nc_cache:
        return _nc_cache[nt]
    nc = bacc.Bacc()
    feats = nc.declare_dram_parameter(
        "features", [nt * P * S, D], mybir.dt.float32, isOutput=False
    )
    labels = nc.declare_dram_parameter("labels", [P, nt], mybir.dt.int32, isOutput=False)
    centers = nc.declare_dram_parameter(
        "centers", [C, D], mybir.dt.float32, isOutput=False
    )
    out = nc.declare_dram_parameter("out", [P, nt], mybir.dt.float32, isOutput=True)

    # tile t, partition p, slot s <- padded feature row (t*128 + p)*16 + s
    fview = feats[:].rearrange("(t p s) d -> t p s d", p=P, s=S)

    with tile.TileContext(nc) as tc:
        with (
            tc.tile_pool(name="lab", bufs=1) as lab_pool,
            tc.tile_pool(name="f", bufs=4) as f_pool,
            tc.tile_pool(name="c", bufs=4) as c_pool,
            tc.tile_pool(name="acc", bufs=1) as acc_pool,
        ):
            lab = lab_pool.tile([P, nt], mybir.dt.int32)
            nc.sync.dma_start(out=lab[:], in_=labels[:])
            acc = acc_pool.tile([P, nt], mybir.dt.float32)
            for t in range(nt):
                f_t = f_pool.tile([P, S * D], mybir.dt.float32)
                nc.sync.dma_start(
                    out=f_t[:].rearrange("p (s d) -> p s d", s=S), in_=fview[t]
                )
                c_s = c_pool.tile([P, D], mybir.dt.float32)
                nc.gpsimd.indirect_dma_start(
                    out=c_s[:],
                    out_offset=None,
                    in_=centers[:],
                    in_offset=bass.IndirectOffsetOnAxis(ap=lab[:, t : t + 1], axis=0),
                )
                c_b = (
                    c_s[:]
                    .rearrange("p (s d) -> p s d", s=1)
                    .to_broadcast([P, S, D])
                )
                nc.vector.tensor_tensor(
                    out=f_t[:].rearrange("p (s d) -> p s d", s=S),
                    in0=f_t[:].rearrange("p (s d) -> p s d", s=S),
                    in1=c_b,
                    op=mybir.AluOpType.subtract,
                )
                nc.scalar.activation(
                    out=f_t[:],
                    in_=f_t[:],
                    func=mybir.ActivationFunctionType.Square,
                    accum_out=acc[:, t : t + 1],
                )
            nc.sync.dma_start(out=out[:], in_=acc[:])
    nc.finalize()
    _nc_cache[nt] = nc
    return nc


def _prepare(features, centers, labels):
    """Sort rows by label into padded 16-row blocks; returns per-core maps + nt."""
    features = np.ascontiguousarray(np.asarray(features), dtype=np.float32)
    centers = np.ascontiguousarray(np.asarray(centers), dtype=np.float32)
    labels = np.asarray(labels).astype(np.int32)

    counts = np.bincount(labels, minlength=C)          # [C]
    nblocks = -(-counts // S)                          # ceil(n_c / S) per class
    nb = int(nblocks.sum())
    group = N_CORES * P                                # blocks per tile-row across chip
    nb_pad = -(-nb // group) * group
    nt = nb_pad // group

    # block labels, in sorted-class order; pad blocks use class 0
    block_labels = np.zeros(nb_pad, dtype=np.int32)
    block_labels[:nb] = np.repeat(np.arange(C, dtype=np.int32), nblocks)

    # every padded slot starts as its block's center row -> contributes 0
    fpad = centers[block_labels].repeat(S, axis=0).reshape(nb_pad * S, D)

    # scatter the real rows into their slots
    order = np.argsort(labels)
    labels_sorted = labels[order]
    class_row_start = np.concatenate(([0], np.cumsum(counts)[:-1]))
    class_slot_start = S * np.concatenate(([0], np.cumsum(nblocks)[:-1]))
    rank = np.arange(B) - class_row_start[labels_sorted]
    dst = class_slot_start[labels_sorted] + rank
    fpad[dst] = features[order]

    rows_core = nt * P * S
    maps = []
    for k in range(N_CORES):
        fs = fpad[k * rows_core : (k + 1) * rows_core]
        # labW[p, t] = block_labels[(k*nt + t)*128 + p]
        lw = np.ascontiguousarray(
            block_labels[k * nt * P : (k + 1) * nt * P].reshape(nt, P).T
        )
        maps.append({"features": fs, "labels": lw, "centers": centers})
    return maps, nt


def run(features, centers, labels, trace=False):
    """Run on 8 cores; returns (loss_scalar, BassKernelResults)."""
    maps, nt = _prepare(features, centers, labels)
    nc = _build(nt)
    res = run_bass_kernel_spmd(
        nc, maps, core_ids=list(range(N_CORES)), trace=trace
    )
    total = 0.0
    for r in res.results:
        total += float(np.asarray(r["out"]).astype(np.float64).sum())
    return np.float32(total / B), res


def kernel(features, centers, labels):
    loss, _ = run(features, centers, labels)
    return loss
